# revision 7
# baseline (speedup 1.0000x reference)
"""Causal self-attention (B=2, S=2048, D=1024, H=16, Dh=64) on 8 trn2 cores.

Sharding: data-parallel over batch (2 groups of 4 cores) x tensor-parallel over
heads (4 heads/core). Each core computes its heads' attention and a partial
c_proj product; the host sums the 4 partials per batch and adds b_proj.

Design notes (cost-model driven):
  - The PE p-state ramp (0.65/1.2/2.4 GHz) only reaches full clock after 3us
    of continuous execution, and any stall resets it. The kernel is built as
    one dense PE stream: warmup matmuls cover the initial DMA window, QKV
    projection for s-chunk c is fused with attention for q-block c, and PV /
    c_proj work for iteration i-1 is interleaved into the scores stream of
    iteration i via generators.
  - Scores run as fp8e4m3 DoubleRow matmuls (0.5 cyc/row): q/k are stored
    [32 partitions, 2, S] per head so one DR matmul contracts all 64 dh.
    Everything else is bf16 (fp8 elsewhere breaks the 2e-2 error budget).
  - Fine-grained causality: diagonal 128x128 sub-blocks only, exact-length
    PV chains per 128-wide q block, single [128,128] triangle mask.
  - Softmax denominator rides as a ones-column in V (psum row 64); the
    normalize is transpose-free: DVE reciprocal of the denominator row,
    GPSIMD partition_broadcast, then a row-scale multiply into O^T.
  - Output y and all inputs are bf16; host sums the 4 partials in fp32.
"""

import os
import sys

for _p in ("/opt/trn_rl_repo", "/root/.axon_site/_ro/trn_rl_repo"):
    if os.path.isdir(_p) and _p not in sys.path:
        sys.path.insert(0, _p)

import numpy as np
import ml_dtypes

import concourse.bacc as bacc
import concourse.tile as tile
from concourse import mybir
from concourse.bass_utils import run_bass_kernel_spmd

F32 = mybir.dt.float32
BF16 = mybir.dt.bfloat16
FP8 = mybir.dt.float8e4
DR = mybir.MatmulPerfMode.DoubleRow
EXP = mybir.ActivationFunctionType.Exp

B, S, D, H, DH = 2, 2048, 1024, 16, 64
HC = 4          # heads per core
EV = 256        # v cols per core
ND = D // 128   # 8 d-tiles
NS = S // 128   # 16 k-tiles of 128
NQ = S // 512   # 4 q-blocks of 512
N_WARMUP = 34   # junk matmuls covering the initial DMA window


def build_nc():
    nc = bacc.Bacc("TRN2", target_bir_lowering=False, debug=False)

    xT = nc.dram_tensor("xT", [D, S], BF16, kind="ExternalInput").ap()
    wqk = nc.dram_tensor("wqk", [D, 512], BF16, kind="ExternalInput").ap()
    bqk = nc.dram_tensor("bqk", [128, 4], F32, kind="ExternalInput").ap()
    wv = nc.dram_tensor("wv", [D, EV], BF16, kind="ExternalInput").ap()
    bv = nc.dram_tensor("bv", [1, EV], BF16, kind="ExternalInput").ap()
    wp = nc.dram_tensor("wp", [128, 2 * D], BF16, kind="ExternalInput").ap()
    tri = nc.dram_tensor("tri", [128, 128], BF16, kind="ExternalInput").ap()
    y = nc.dram_tensor("y", [S, D], BF16, kind="ExternalOutput").ap()

    with tile.TileContext(nc) as tc:
        _emit(nc, tc, xT, wqk, bqk, wv, bv, wp, tri, y)
    nc.compile()
    return nc


def _drive(gen, n):
    for _ in range(n):
        if next(gen, None) is None:
            return False
    return True


def _emit(nc, tc, xT, wqk, bqk, wv, bv, wp, tri, y):
    from contextlib import ExitStack

    with ExitStack() as top:
        consts = top.enter_context(tc.tile_pool(name="consts", bufs=1))
        acts = top.enter_context(tc.tile_pool(name="acts", bufs=1))
        pb = top.enter_context(tc.tile_pool(name="pb", bufs=2, space="PSUM"))
        psc_pool = top.enter_context(tc.tile_pool(name="psc", bufs=2, space="PSUM"))
        po_pool = top.enter_context(tc.tile_pool(name="po", bufs=1, space="PSUM"))
        small = top.enter_context(tc.tile_pool(name="small", bufs=4))
        rb_pool = top.enter_context(tc.tile_pool(name="rb", bufs=2))
        ys_pool = top.enter_context(tc.tile_pool(name="ys", bufs=3))

        # ---- consts (memsets on gpsimd; tiny DMAs first in queue) ----
        ones1 = consts.tile([1, 128], BF16)
        nc.gpsimd.memset(ones1[:], 1.0)
        wm = consts.tile([128, 512], BF16)
        nc.gpsimd.memset(wm[:], 0.0)
        bqk_sb = consts.tile([128, 4], F32)
        nc.sync.dma_start(bqk_sb[:], bqk[:])
        bv_sb = consts.tile([1, EV], BF16)
        nc.sync.dma_start(bv_sb[:], bv[:])
        tri_sb = consts.tile([128, 128], BF16)
        nc.sync.dma_start(tri_sb[:], tri[:])

        # ---- big SBUF residents ----
        wqk_sb = acts.tile([128, ND, 512], BF16, tag="wqk")
        wv_sb = acts.tile([128, ND, EV], BF16, tag="wv")
        wp_sb = acts.tile([128, 2, D], BF16, tag="wp")
        xt_sb = acts.tile([128, ND, S], BF16, tag="xt")
        q8 = acts.tile([128, 2, S], FP8, tag="q8")
        k8 = acts.tile([128, 2, S], FP8, tag="k8")
        vhat = acts.tile([128, HC, NS, DH + 1], BF16, tag="vhat")
        nc.gpsimd.memset(vhat[:, :, :, DH:DH + 1], 1.0)
        # phat[:, hp, h, kt, q_local]
        phat = acts.tile([128, 2, 2, NS, 512], BF16, tag="phat")
        ot = acts.tile([128, 2, S], BF16, tag="ot")

        # ---- input DMA queue (ordered by first use) ----
        wqk_r = wqk.rearrange("(t p) c -> p t c", p=128)
        xT_r = xT.rearrange("(t p) s -> p t s", p=128)
        wv_r = wv.rearrange("(t p) c -> p t c", p=128)
        nc.sync.dma_start(wqk_sb[:], wqk_r)
        nc.sync.dma_start(xt_sb[:, :, 0:512], xT_r[:, :, 0:512])
        nc.sync.dma_start(wv_sb[:], wv_r)
        nc.sync.dma_start(wp_sb[:], wp.rearrange("p (t c) -> p t c", t=2))
        for c in range(1, NQ):
            nc.sync.dma_start(xt_sb[:, :, 512 * c:512 * (c + 1)],
                              xT_r[:, :, 512 * c:512 * (c + 1)])

        # ---- warmup: keep PE busy (and ramping) while inputs land ----
        junk = pb.tile([128, 512], F32, tag="pb", name="junk")
        for _ in range(N_WARMUP):
            nc.tensor.matmul(junk[:], wm[:, 0:128], wm[:], start=True, stop=True)

        def pv_gen(qt, hp):
            """PV + normalize for (qt, hp); c_proj + y out for qt when hp==1."""
            po = po_pool.tile([65, 2, 4, 128], F32, tag="po", name="po")
            for h in range(2):
                hg = 2 * hp + h
                for j in range(4):
                    nk = 4 * qt + j + 1
                    for kt in range(nk):
                        nc.tensor.matmul(
                            po[:, h, j, :],
                            vhat[:, hg, kt, :],
                            phat[:, hp, h, kt, 128 * j:128 * (j + 1)],
                            start=(kt == 0), stop=(kt == nk - 1),
                        )
                        yield
                rd = small.tile([1, 4, 128], F32, tag="rd", name="rd")
                nc.vector.reciprocal(rd[:], po[64:65, h, :, :])
                yield
                rb = rb_pool.tile([64, 4, 128], F32, tag="rb", name="rb")
                nc.gpsimd.partition_broadcast(rb[:], rd[:])
                yield
                ot_v = ot[64 * h:64 * (h + 1), hp, 512 * qt:512 * (qt + 1)]
                nc.vector.tensor_mul(
                    ot_v.rearrange("p (j q) -> p j q", j=4), po[0:64, h, :, :], rb[:]
                )
                yield
            if hp == 1:
                for s4 in range(4):
                    st = 4 * qt + s4
                    ys = ys_pool.tile([128, D], BF16, tag="ys", name="ys")
                    for nt in range(2):
                        py = pb.tile([128, 512], F32, tag="pb", name="py")
                        for ft in range(2):
                            nc.tensor.matmul(
                                py[:],
                                ot[:, ft, 128 * st:128 * (st + 1)],
                                wp_sb[:, ft, 512 * nt:512 * (nt + 1)],
                                start=(ft == 0), stop=(ft == 1),
                            )
                        yield
                        nc.vector.tensor_copy(ys[:, 512 * nt:512 * (nt + 1)], py[:])
                        yield
                    nc.sync.dma_start(y[128 * st:128 * (st + 1), :], ys[:])
                    yield

        prev = iter(())
        for qt in range(NQ):
            # ---------- phase A chunk qt: QKV projection for s-block ----------
            for t in range(4):  # 0=(Q,i0) 1=(Q,i1) 2=(K,i0) 3=(K,i1)
                ps = pb.tile([128, 512], F32, tag="pb", name=f"qk{t}")
                for dt in range(ND):
                    nc.tensor.matmul(
                        ps[:],
                        wqk_sb[:, dt, 128 * t:128 * (t + 1)],
                        xt_sb[:, dt, 512 * qt:512 * (qt + 1)],
                        start=(dt == 0), stop=(dt == ND - 1),
                    )
                _drive(prev, 3)
                dest = q8 if t < 2 else k8
                nc.vector.tensor_scalar_add(
                    dest[:, t % 2, 512 * qt:512 * (qt + 1)], ps[:], bqk_sb[:, t:t + 1]
                )
            for s4 in range(4):
                st = 4 * qt + s4
                ps = pb.tile([128, 512], F32, tag="pb", name="v")
                pv = ps[:, 0:EV]
                for dt in range(ND):
                    nc.tensor.matmul(
                        pv, xt_sb[:, dt, 128 * st:128 * (st + 1)], wv_sb[:, dt, :],
                        start=(dt == 0), stop=False,
                    )
                nc.tensor.matmul(pv, ones1[:], bv_sb[:], start=False, stop=True)
                _drive(prev, 2)
                nc.vector.tensor_copy(
                    vhat[:, :, st, 0:DH], pv.rearrange("p (h e) -> p h e", h=HC)
                )

            # ---------- phase B: scores + exp (+mask) for (qt, hp) ----------
            for hp in range(2):
                for h in range(2):
                    hg = 2 * hp + h
                    bp = 32 * hg
                    nfull = 4 * qt
                    kt = 0
                    while kt < nfull:  # full k-tiles, exp'd in pairs
                        pair = min(2, nfull - kt)
                        psc = psc_pool.tile([128, 2, 512], F32, tag="psc", name="psc")
                        for k2 in range(pair):
                            nc.tensor.matmul(
                                psc[:, k2, :],
                                k8[bp:bp + 32, :, 128 * (kt + k2):128 * (kt + k2 + 1)],
                                q8[bp:bp + 32, :, 512 * qt:512 * (qt + 1)],
                                start=True, stop=True, perf_mode=DR,
                                tile_position=(bp, 0),
                            )
                            _drive(prev, 3)
                        nc.scalar.activation(
                            phat[:, hp, h, kt:kt + pair, :], psc[:, 0:pair, :],
                            EXP, scale=0.125,
                        )
                        kt += pair
                    for c in range(4):  # diagonal k-tiles, exact causal range
                        ktd = 4 * qt + c
                        npc = 512 - 128 * c
                        psc = psc_pool.tile([128, 2, 512], F32, tag="psc", name="pscd")
                        nc.tensor.matmul(
                            psc[:, 0, 128 * c:512],
                            k8[bp:bp + 32, :, 128 * ktd:128 * (ktd + 1)],
                            q8[bp:bp + 32, :, 512 * qt + 128 * c:512 * (qt + 1)],
                            start=True, stop=True, perf_mode=DR,
                            tile_position=(bp, 0),
                        )
                        _drive(prev, 3)
                        nc.scalar.activation(
                            phat[:, hp, h, ktd, 128 * c:512], psc[:, 0, 128 * c:512],
                            EXP, scale=0.125,
                        )
                        msl = phat[:, hp, h, ktd, 128 * c:128 * (c + 1)]
                        nc.gpsimd.tensor_mul(msl, msl, tri_sb[:])
                for _ in prev:
                    pass
                prev = pv_gen(qt, hp)
        for _ in prev:
            pass


_NC = None


def _get_nc():
    global _NC
    if _NC is None:
        _NC = build_nc()
    return _NC


def _make_tri():
    i = np.arange(128)[:, None]
    j = np.arange(128)[None, :]
    return (i <= j).astype(ml_dtypes.bfloat16)


def _in_maps(x, W_attn, b_attn, W_proj):
    bf = ml_dtypes.bfloat16
    tri = _make_tri()
    maps = []
    for cidx in range(8):
        b, g = cidx // 4, cidx % 4
        wqk_c = np.empty((D, 512), np.float32)
        bqk_c = np.empty((128, 4), np.float32)
        for t in range(4):
            off = 0 if t < 2 else D
            i = t % 2
            for hgl in range(4):
                hgg = 4 * g + hgl
                src = off + 64 * hgg + 32 * i
                wqk_c[:, 128 * t + 32 * hgl:128 * t + 32 * (hgl + 1)] = \
                    W_attn[:, src:src + 32]
                bqk_c[32 * hgl:32 * (hgl + 1), t] = b_attn[src:src + 32]
        wv_c = W_attn[:, 2 * D + EV * g:2 * D + EV * (g + 1)]
        bv_c = b_attn[2 * D + EV * g:2 * D + EV * (g + 1)].reshape(1, EV)
        wp_c = np.ascontiguousarray(
            W_proj[EV * g:EV * (g + 1), :].reshape(2, 128, D)
            .transpose(1, 0, 2).reshape(128, 2 * D))
        maps.append({
            "xT": np.ascontiguousarray(x[b].T).astype(bf),
            "wqk": wqk_c.astype(bf),
            "bqk": np.ascontiguousarray(bqk_c),
            "wv": np.ascontiguousarray(wv_c).astype(bf),
            "bv": bv_c.astype(bf),
            "wp": wp_c.astype(bf),
            "tri": tri,
        })
    return maps


def _gather(results, b_proj):
    y = np.empty((B, S, D), np.float32)
    for b in range(B):
        acc = results[4 * b]["y"].astype(np.float32)
        for g in range(1, 4):
            acc = acc + results[4 * b + g]["y"].astype(np.float32)
        y[b] = acc + b_proj[None, :]
    return y


def run(x, W_attn, b_attn, W_proj, b_proj, trace=False):
    x = np.asarray(x, np.float32)
    W_attn = np.asarray(W_attn, np.float32)
    b_attn = np.asarray(b_attn, np.float32)
    W_proj = np.asarray(W_proj, np.float32)
    b_proj = np.asarray(b_proj, np.float32)
    nc = _get_nc()
    res = run_bass_kernel_spmd(nc, _in_maps(x, W_attn, b_attn, W_proj),
                               core_ids=list(range(8)), trace=trace)
    return _gather(res.results, b_proj), res


def kernel(x, W_attn, b_attn, W_proj, b_proj):
    out, _ = run(x, W_attn, b_attn, W_proj, b_proj)
    return out


# revision 13
# speedup vs baseline: 1.9791x; 1.9791x over previous
"""Causal self-attention (B=2, S=2048, D=1024, H=16, Dh=64) on 8 trn2 cores.

Sharding: data-parallel over batch (2 groups of 4 cores) x tensor-parallel over
heads (4 heads/core). Each core computes its heads' attention and a partial
c_proj product; the host sums the 4 partials per batch and adds b_proj.

Design notes (cost-model driven):
  - The PE p-state ramp (0.65/1.2/2.4 GHz) only reaches full clock after 3us
    of continuous execution, and any stall resets it. The kernel is built as
    one dense PE stream: warmup matmuls cover the initial DMA window, QKV
    projection for s-chunk c is fused with attention for q-block c, and PV /
    c_proj work for iteration i-1 is interleaved into the scores stream of
    iteration i via generators.
  - Scores run as fp8e4m3 DoubleRow matmuls (0.5 cyc/row): q/k are stored
    [32 partitions, 2, S] per head so one DR matmul contracts all 64 dh.
    Everything else is bf16 (fp8 elsewhere breaks the 2e-2 error budget).
  - Fine-grained causality: diagonal 128x128 sub-blocks only, exact-length
    PV chains per 128-wide q block, single [128,128] triangle mask.
  - Softmax denominator rides as a ones-column in V (psum row 64); the
    normalize is transpose-free: DVE reciprocal of the denominator row,
    GPSIMD partition_broadcast, then a row-scale multiply into O^T.
  - Output y and all inputs are bf16; host sums the 4 partials in fp32.
"""

import os
import sys

for _p in ("/opt/trn_rl_repo", "/root/.axon_site/_ro/trn_rl_repo"):
    if os.path.isdir(_p) and _p not in sys.path:
        sys.path.insert(0, _p)

import numpy as np
import ml_dtypes

import concourse.bacc as bacc
import concourse.tile as tile
from concourse import mybir
from concourse.bass_utils import run_bass_kernel_spmd

F32 = mybir.dt.float32
BF16 = mybir.dt.bfloat16
FP8 = mybir.dt.float8e4
DR = mybir.MatmulPerfMode.DoubleRow
EXP = mybir.ActivationFunctionType.Exp

B, S, D, H, DH = 2, 2048, 1024, 16, 64
HC = 4          # heads per core
EV = 256        # v cols per core
ND = D // 128   # 8 d-tiles
NS = S // 128   # 16 k-tiles of 128
NQ = S // 512   # 4 q-blocks of 512
N_WARMUP = 34   # junk matmuls covering the initial DMA window


def build_nc():
    nc = bacc.Bacc("TRN2", target_bir_lowering=False, debug=False)

    xT = nc.dram_tensor("xT", [D, S], BF16, kind="ExternalInput").ap()
    wqk = nc.dram_tensor("wqk", [D, 512], BF16, kind="ExternalInput").ap()
    bqk = nc.dram_tensor("bqk", [128, 4], F32, kind="ExternalInput").ap()
    wv = nc.dram_tensor("wv", [D, EV], BF16, kind="ExternalInput").ap()
    bv = nc.dram_tensor("bv", [1, EV], BF16, kind="ExternalInput").ap()
    wp = nc.dram_tensor("wp", [128, 2 * D], BF16, kind="ExternalInput").ap()
    tri = nc.dram_tensor("tri", [128, 128], BF16, kind="ExternalInput").ap()
    y = nc.dram_tensor("y", [S, D], BF16, kind="ExternalOutput").ap()

    with tile.TileContext(nc) as tc:
        _emit(nc, tc, xT, wqk, bqk, wv, bv, wp, tri, y)
    nc.compile()
    return nc


def _drive(gen, n):
    for _ in range(n):
        if next(gen, None) is None:
            return False
    return True


def _emit(nc, tc, xT, wqk, bqk, wv, bv, wp, tri, y):
    from contextlib import ExitStack

    with ExitStack() as top:
        consts = top.enter_context(tc.tile_pool(name="consts", bufs=1))
        acts = top.enter_context(tc.tile_pool(name="acts", bufs=1))
        pb = top.enter_context(tc.tile_pool(name="pb", bufs=2, space="PSUM"))
        psc_pool = top.enter_context(tc.tile_pool(name="psc", bufs=2, space="PSUM"))
        po_pool = top.enter_context(tc.tile_pool(name="po", bufs=2, space="PSUM"))
        small = top.enter_context(tc.tile_pool(name="small", bufs=4))
        rb_pool = top.enter_context(tc.tile_pool(name="rb", bufs=2))
        ys_pool = top.enter_context(tc.tile_pool(name="ys", bufs=3))

        # ---- consts (memsets on gpsimd; tiny DMAs first in queue) ----
        ones1 = consts.tile([1, 128], BF16)
        nc.gpsimd.memset(ones1[:], 1.0)
        wm = consts.tile([128, 512], BF16)
        nc.gpsimd.memset(wm[:], 0.0)
        bqk_sb = consts.tile([128, 4], F32)
        nc.sync.dma_start(bqk_sb[:], bqk[:])
        bv_sb = consts.tile([1, EV], BF16)
        nc.sync.dma_start(bv_sb[:], bv[:])
        tri_sb = consts.tile([128, 128], BF16)
        nc.sync.dma_start(tri_sb[:], tri[:])

        # ---- big SBUF residents ----
        wqk_sb = acts.tile([128, ND, 512], BF16, tag="wqk")
        wv_sb = acts.tile([128, ND, EV], BF16, tag="wv")
        wp_sb = acts.tile([128, 2, D], BF16, tag="wp")
        xt_sb = acts.tile([128, ND, S], BF16, tag="xt")
        q8 = acts.tile([128, 2, S], FP8, tag="q8")
        k8 = acts.tile([128, 2, S], FP8, tag="k8")
        vhat = acts.tile([128, HC, NS, DH + 1], BF16, tag="vhat")
        nc.gpsimd.memset(vhat[:, :, :, DH:DH + 1], 1.0)
        # phat[:, hp, h, kt, q_local]
        phat = acts.tile([128, 2, 2, NS, 512], BF16, tag="phat")
        ot = acts.tile([128, 2, S], BF16, tag="ot")

        # ---- input DMA queue (ordered by first use) ----
        wqk_r = wqk.rearrange("(t p) c -> p t c", p=128)
        xT_r = xT.rearrange("(t p) s -> p t s", p=128)
        wv_r = wv.rearrange("(t p) c -> p t c", p=128)
        nc.sync.dma_start(wqk_sb[:], wqk_r)
        nc.sync.dma_start(xt_sb[:, :, 0:512], xT_r[:, :, 0:512])
        nc.sync.dma_start(wv_sb[:], wv_r)
        nc.sync.dma_start(wp_sb[:], wp.rearrange("p (t c) -> p t c", t=2))
        for c in range(1, NQ):
            nc.sync.dma_start(xt_sb[:, :, 512 * c:512 * (c + 1)],
                              xT_r[:, :, 512 * c:512 * (c + 1)])

        # ---- warmup: keep PE busy (and ramping) while inputs land ----
        junk = pb.tile([128, 512], F32, tag="pb", name="junk")
        for _ in range(N_WARMUP):
            nc.tensor.matmul(junk[:], wm[:, 0:128], wm[:], start=True, stop=True)

        def pv_gen(qt, hp):
            """PV + normalize for (qt, hp); c_proj + y out for qt when hp==1."""
            for h in range(2):
                hg = 2 * hp + h
                po = po_pool.tile([65, 4, 128], F32, tag="po", name="po")
                for j in range(4):
                    nk = 4 * qt + j + 1
                    for kt in range(nk):
                        nc.tensor.matmul(
                            po[:, j, :],
                            vhat[:, hg, kt, :],
                            phat[:, hp, h, kt, 128 * j:128 * (j + 1)],
                            start=(kt == 0), stop=(kt == nk - 1),
                        )
                        yield
                rd = small.tile([1, 4, 128], F32, tag="rd", name="rd")
                nc.vector.tensor_copy(rd[:], po[64:65, :, :])
                yield
                rr = small.tile([1, 4, 128], F32, tag="rr", name="rr")
                nc.vector.reciprocal_approx_fast(rr[:], rd[:])
                yield
                rb = rb_pool.tile([64, 4, 128], F32, tag="rb", name="rb")
                nc.gpsimd.partition_broadcast(rb[:], rr[:])
                yield
                ot_v = ot[64 * h:64 * (h + 1), hp, 512 * qt:512 * (qt + 1)]
                nc.vector.tensor_mul(
                    ot_v.rearrange("p (j q) -> p j q", j=4), po[0:64, :, :], rb[:]
                )
                yield
            if hp == 1:
                for s4 in range(4):
                    st = 4 * qt + s4
                    ys = ys_pool.tile([128, D], BF16, tag="ys", name="ys")
                    for nt in range(2):
                        py = pb.tile([128, 512], F32, tag="pb", name="py")
                        for ft in range(2):
                            nc.tensor.matmul(
                                py[:],
                                ot[:, ft, 128 * st:128 * (st + 1)],
                                wp_sb[:, ft, 512 * nt:512 * (nt + 1)],
                                start=(ft == 0), stop=(ft == 1),
                            )
                        yield
                        nc.vector.tensor_copy(ys[:, 512 * nt:512 * (nt + 1)], py[:])
                        yield
                    nc.sync.dma_start(y[128 * st:128 * (st + 1), :], ys[:])
                    yield

        prev = iter(())
        for qt in range(NQ):
            # ---------- phase A chunk qt: QKV projection for s-block ----------
            for t in range(4):  # 0=(Q,i0) 1=(Q,i1) 2=(K,i0) 3=(K,i1)
                ps = pb.tile([128, 512], F32, tag="pb", name=f"qk{t}")
                for dt in range(ND):
                    nc.tensor.matmul(
                        ps[:],
                        wqk_sb[:, dt, 128 * t:128 * (t + 1)],
                        xt_sb[:, dt, 512 * qt:512 * (qt + 1)],
                        start=(dt == 0), stop=(dt == ND - 1),
                    )
                _drive(prev, 3)
                dest = q8 if t < 2 else k8
                nc.vector.tensor_scalar_add(
                    dest[:, t % 2, 512 * qt:512 * (qt + 1)], ps[:], bqk_sb[:, t:t + 1]
                )
            for s4 in range(4):
                st = 4 * qt + s4
                ps = pb.tile([128, 512], F32, tag="pb", name="v")
                pv = ps[:, 0:EV]
                for dt in range(ND):
                    nc.tensor.matmul(
                        pv, xt_sb[:, dt, 128 * st:128 * (st + 1)], wv_sb[:, dt, :],
                        start=(dt == 0), stop=False,
                    )
                nc.tensor.matmul(pv, ones1[:], bv_sb[:], start=False, stop=True)
                _drive(prev, 2)
                nc.vector.tensor_copy(
                    vhat[:, :, st, 0:DH], pv.rearrange("p (h e) -> p h e", h=HC)
                )

            # ---------- phase B: scores + exp (+mask) for (qt, hp) ----------
            for hp in range(2):
                for h in range(2):
                    hg = 2 * hp + h
                    bp = 32 * hg
                    nfull = 4 * qt
                    kt = 0
                    while kt < nfull:  # full k-tiles, exp'd in pairs
                        pair = min(2, nfull - kt)
                        psc = psc_pool.tile([128, 2, 512], F32, tag="psc", name="psc")
                        for k2 in range(pair):
                            nc.tensor.matmul(
                                psc[:, k2, :],
                                k8[bp:bp + 32, :, 128 * (kt + k2):128 * (kt + k2 + 1)],
                                q8[bp:bp + 32, :, 512 * qt:512 * (qt + 1)],
                                start=True, stop=True, perf_mode=DR,
                                tile_position=(bp, 0),
                            )
                            _drive(prev, 3)
                        nc.scalar.activation(
                            phat[:, hp, h, kt:kt + pair, :], psc[:, 0:pair, :],
                            EXP, scale=0.125,
                        )
                        kt += pair
                    for c in range(4):  # diagonal k-tiles, exact causal range
                        ktd = 4 * qt + c
                        npc = 512 - 128 * c
                        psc = psc_pool.tile([128, 2, 512], F32, tag="psc", name="pscd")
                        nc.tensor.matmul(
                            psc[:, 0, 128 * c:512],
                            k8[bp:bp + 32, :, 128 * ktd:128 * (ktd + 1)],
                            q8[bp:bp + 32, :, 512 * qt + 128 * c:512 * (qt + 1)],
                            start=True, stop=True, perf_mode=DR,
                            tile_position=(bp, 0),
                        )
                        _drive(prev, 3)
                        nc.scalar.activation(
                            phat[:, hp, h, ktd, 128 * c:512], psc[:, 0, 128 * c:512],
                            EXP, scale=0.125,
                        )
                        msl = phat[:, hp, h, ktd, 128 * c:128 * (c + 1)]
                        nc.vector.tensor_mul(msl, msl, tri_sb[:])
                for _ in prev:
                    pass
                prev = pv_gen(qt, hp)
        for _ in prev:
            pass


_NC = None


def _get_nc():
    global _NC
    if _NC is None:
        _NC = build_nc()
    return _NC


def _make_tri():
    i = np.arange(128)[:, None]
    j = np.arange(128)[None, :]
    return (i <= j).astype(ml_dtypes.bfloat16)


def _in_maps(x, W_attn, b_attn, W_proj):
    bf = ml_dtypes.bfloat16
    tri = _make_tri()
    maps = []
    for cidx in range(8):
        b, g = cidx // 4, cidx % 4
        wqk_c = np.empty((D, 512), np.float32)
        bqk_c = np.empty((128, 4), np.float32)
        for t in range(4):
            off = 0 if t < 2 else D
            i = t % 2
            for hgl in range(4):
                hgg = 4 * g + hgl
                src = off + 64 * hgg + 32 * i
                wqk_c[:, 128 * t + 32 * hgl:128 * t + 32 * (hgl + 1)] = \
                    W_attn[:, src:src + 32]
                bqk_c[32 * hgl:32 * (hgl + 1), t] = b_attn[src:src + 32]
        wv_c = W_attn[:, 2 * D + EV * g:2 * D + EV * (g + 1)]
        bv_c = b_attn[2 * D + EV * g:2 * D + EV * (g + 1)].reshape(1, EV)
        wp_c = np.ascontiguousarray(
            W_proj[EV * g:EV * (g + 1), :].reshape(2, 128, D)
            .transpose(1, 0, 2).reshape(128, 2 * D))
        maps.append({
            "xT": np.ascontiguousarray(x[b].T).astype(bf),
            "wqk": wqk_c.astype(bf),
            "bqk": np.ascontiguousarray(bqk_c),
            "wv": np.ascontiguousarray(wv_c).astype(bf),
            "bv": bv_c.astype(bf),
            "wp": wp_c.astype(bf),
            "tri": tri,
        })
    return maps


def _gather(results, b_proj):
    y = np.empty((B, S, D), np.float32)
    for b in range(B):
        acc = results[4 * b]["y"].astype(np.float32)
        for g in range(1, 4):
            acc = acc + results[4 * b + g]["y"].astype(np.float32)
        y[b] = acc + b_proj[None, :]
    return y


def run(x, W_attn, b_attn, W_proj, b_proj, trace=False):
    x = np.asarray(x, np.float32)
    W_attn = np.asarray(W_attn, np.float32)
    b_attn = np.asarray(b_attn, np.float32)
    W_proj = np.asarray(W_proj, np.float32)
    b_proj = np.asarray(b_proj, np.float32)
    nc = _get_nc()
    res = run_bass_kernel_spmd(nc, _in_maps(x, W_attn, b_attn, W_proj),
                               core_ids=list(range(8)), trace=trace)
    return _gather(res.results, b_proj), res


def kernel(x, W_attn, b_attn, W_proj, b_proj):
    out, _ = run(x, W_attn, b_attn, W_proj, b_proj)
    return out
